# revision 17
# baseline (speedup 1.0000x reference)
"""AVWGCN graph-conv kernel for 8 Trainium2 NeuronCores (node-parallel, fp8).

out[b,n,o] = sum_ki xg[b,n,ki] * W[n,ki,o] + bias[n,o]
  xg = concat([x, S@x]), S = softmax(relu(E E^T)) row-wise
  W[n] = sum_d E[n,d] Wp[d], bias = E @ bias_pool

All E-derived quantities are static "weights" and are packed on the host:
  A8 = fp8e4(exp(relu(E E^T)) * 64 / colsum)   (column-normalized adjacency,
       x64 scale keeps flat columns out of the fp8 denormal floor; the 1/64
       is folded into the PSUM-eviction scale)
  W  = E @ weights_pool (bf16), ebias = E @ bias_pool (bf16, replicated)
  x is quantized to fp8e4 and laid out in DoubleRow-paired (mcp, j) order.

Device, per core (625 own nodes, padded 640; 5120 padded source nodes):
  phase 2: per 8-batch chunk: xg1 = A8^T x8 via fp8 DoubleRow chains
           (20 matmuls of 256-wide contraction, 512-wide free), evicted
           *1/64 to bf16 SBUF-resident xgs; xg0 (= own x) DMA'd from host.
  phase 3: per 128-node block: PE-transpose xgs -> xgt (ki,b,n); per-node
           matmuls 2-at-a-time via column tiling (tile_position (0,0)/(0,64));
           bias added during PSUM eviction (scalar_tensor_tensor) and the
           (j,b)-interleaved output un-interleaved on the host.
"""

import sys

sys.path.insert(0, "/opt/trn_rl_repo")

from contextlib import ExitStack

import ml_dtypes
import numpy as np

import concourse.bacc as bacc
import concourse.bass as bass
import concourse.mybir as mybir
import concourse.tile as tile
from concourse.masks import make_identity
from concourse.tile import TileContext

F32 = mybir.dt.float32
BF16 = mybir.dt.bfloat16
F8E4 = mybir.dt.float8e4
AF = mybir.ActivationFunctionType
ALU = mybir.AluOpType
DR = mybir.MatmulPerfMode.DoubleRow

FULL_CFG = dict(B=64, C=64, N=5000, GP=640, M=5120, NCORES=8)
ASCALE = 64.0  # A8 = A_norm * ASCALE; undone at xg1 eviction
import os

SWI = os.environ.get("K_SWI", "0") == "1"  # DoubleRowSwInterleave A8 layout
KMAJOR = os.environ.get("K_KMAJOR", "0") == "1"  # xgs [p, nb, k, b, c] layout


def build_nc(cfg):
    B = cfg["B"]  # batches
    C = cfg["C"]  # channels (64)
    GP = cfg["GP"]  # own nodes per core, padded (mult of 128)
    M = cfg["M"]  # total source nodes, padded (mult of 256)
    MCP = M // 256  # DoubleRow m-pair chunks
    NB = GP // 128
    BG = 8  # batches per chunk (512 free)
    BCC = B // BG
    KI = 2 * C  # 128

    nc = bacc.Bacc("TRN2", target_bir_lowering=False)
    x8d = nc.dram_tensor("x8d", [128, BCC, MCP, 2, BG * C], F8E4, kind="ExternalInput")
    a8d = nc.dram_tensor(
        "a8d",
        [128, MCP, NB, 256] if SWI else [128, MCP, 2, GP],
        F8E4,
        kind="ExternalInput",
    )
    xpnd = nc.dram_tensor("xpnd", [128, NB, B, C], BF16, kind="ExternalInput")
    wtd = nc.dram_tensor("wtd", [128, NB, C, 128], BF16, kind="ExternalInput")
    ebd = nc.dram_tensor("ebd", [128, NB, 64, C], BF16, kind="ExternalInput")
    outd = nc.dram_tensor("out_sh", [B, NB, 2, 64, C], BF16, kind="ExternalOutput")

    with TileContext(nc) as tc, ExitStack() as ctx:
        const = ctx.enter_context(tc.tile_pool(name="const", bufs=1))
        ident = const.tile([128, 128], BF16)
        make_identity(nc, ident)

        # xg resident in SBUF: [n_part, nb, b, k, c]
        xgs_p = ctx.enter_context(tc.tile_pool(name="xgs", bufs=1))
        xgs = xgs_p.tile(
            [128, NB, 2, B, C] if KMAJOR else [128, NB, B, 2, C], BF16
        )

        # phase-3 weight/transpose pools created early so prefetch overlaps
        wtp = ctx.enter_context(tc.tile_pool(name="wtp", bufs=2))
        xgtp = ctx.enter_context(tc.tile_pool(name="xgtp", bufs=2))

        # phase-3 PSUM pools created before phase 2 so the leading transposes
        # can overlap the last batch-chunk's chains (pst gets the other 4 banks)
        tps = ctx.enter_context(tc.tile_pool(name="tps", bufs=2, space="PSUM"))
        ops = ctx.enter_context(tc.tile_pool(name="ops", bufs=2, space="PSUM"))

        def emit_transpose_group(nb, xgt2, b4):
            # 4 b's transposed into one PSUM bank, evicted ACT/DVE alternately
            pt = tps.tile([128, 4, 128], BF16, tag="pt")
            for j in range(4):
                b = b4 * 4 + j
                nc.tensor.transpose(
                    pt[:, j, :],
                    xgs[:, nb, :, b, :]
                    if KMAJOR
                    else xgs[:, nb, b, :, :].rearrange("p a b -> p (a b)"),
                    ident,
                )
            # 3:1 ACT/DVE split — DVE also carries the osb evictions
            if b4 % 4 != 3:
                nc.scalar.activation(xgt2[:, b4 * 4 : (b4 + 1) * 4, :], pt, AF.Copy)
            else:
                nc.vector.tensor_copy(xgt2[:, b4 * 4 : (b4 + 1) * 4, :], pt)

        def start_block(nb):
            # allocate + prefetch a node block's weight/transpose tiles
            wt2 = wtp.tile([128, C, 128], BF16, tag="wt2")
            nc.sync.dma_start(wt2, wtd[:, nb, :, :])
            xgt2 = xgtp.tile([128, B, 128], BF16, tag="xgt2")
            return (xgt2, wt2)

        blocks = {}

        # ---- phase 2: xg1 = (A8^T x8) / ASCALE into xgs ----
        # phase-2-only pools (innermost; closed together at the phase boundary
        # so eb/osb pools can reuse their SBUF)
        p2 = ExitStack()
        a8pool = p2.enter_context(tc.tile_pool(name="a8", bufs=1))
        a8s = a8pool.tile([128, MCP, NB, 256] if SWI else [128, MCP, 2, GP], F8E4)
        for q in range(4):
            qs = slice(q * (MCP // 4), (q + 1) * (MCP // 4))
            # scalar = the second HWDGE ring; keeps a8 off the x8 queue
            nc.scalar.dma_start(a8s[:, qs], a8d[:, qs])
        xbp = p2.enter_context(tc.tile_pool(name="xb", bufs=6))
        pst_pool = p2.enter_context(tc.tile_pool(name="pst", bufs=4, space="PSUM"))
        MQ = MCP // 4
        for bcc in range(BCC):
            bsl = slice(bcc * BG, (bcc + 1) * BG)
            xh = []
            for q in range(4):
                xb = xbp.tile([128, MQ, 2, BG * C], F8E4, tag="xb")
                nc.sync.dma_start(xb, x8d[:, bcc, q * MQ : (q + 1) * MQ, :, :])
                xh.append(xb)
            if 1 <= bcc <= NB:
                # k=0 half of xgs (= own x, host-transposed): needed only by
                # phase 3, trickled in through phase-2 DMA slack
                nc.scalar.dma_start(
                    xgs[:, bcc - 1, 0, :, :] if KMAJOR else xgs[:, bcc - 1, :, 0, :],
                    xpnd[:, bcc - 1, :, :],
                )
            for nb in range(NB):
                nsl = slice(nb * 128, (nb + 1) * 128)
                pst = pst_pool.tile([128, BG * C], F32, tag="pst")
                for mcp in range(MCP):
                    nc.tensor.matmul(
                        pst,
                        lhsT=a8s[:, mcp, nb, :] if SWI else a8s[:, mcp, :, nsl],
                        rhs=xh[mcp // MQ][:, mcp % MQ, :, :],
                        start=(mcp == 0),
                        stop=(mcp == MCP - 1),
                        perf_mode=mybir.MatmulPerfMode.DoubleRowSwInterleave
                        if SWI
                        else DR,
                    )
                nc.scalar.activation(
                    xgs[:, nb, 1, bsl, :] if KMAJOR else xgs[:, nb, bsl, 1, :],
                    pst,
                    AF.Copy,
                    scale=1.0 / ASCALE,
                )
                if bcc == BCC - 1 and nb in (1, 2):
                    # xgs[nb-1] is complete: transpose it under the remaining
                    # chains (xgtp has 2 bufs -> at most 2 leading blocks)
                    tnb = nb - 1
                    blocks[tnb] = start_block(tnb)
                    for b4 in range(B // 4):
                        emit_transpose_group(tnb, blocks[tnb][0], b4)
        p2.close()

        # ---- phase 3: per-node matmuls (col-tiled pairs), bias at eviction.
        # Node blocks 0/1 were transposed under phase 2; block nb+1 is
        # transposed interleaved with block nb's node matmuls.
        ebsp = ctx.enter_context(tc.tile_pool(name="ebs", bufs=1))
        ebs = ebsp.tile([128, NB, 64, C], BF16)
        for nb in range(NB):
            nc.scalar.dma_start(ebs[:, nb, :, :], ebd[:, nb, :, :])
        osbp = ctx.enter_context(tc.tile_pool(name="osbp", bufs=2))

        def emit_node_group(nb, xgt2, wt2, osb, g8):
            # 16 nodes (8 col-tiled pairs) per PSUM bank; bias at eviction
            po = ops.tile([128, 8, C], F32, tag="po")
            for pg in range(8):
                n0 = g8 * 16 + pg * 2
                nc.tensor.matmul(
                    po[0:64, pg, :],
                    lhsT=xgt2[:, :, n0],
                    rhs=wt2[:, :, n0],
                    start=True,
                    stop=True,
                    tile_position=(0, 0),
                )
                nc.tensor.matmul(
                    po[64:128, pg, :],
                    lhsT=xgt2[:, :, n0 + 1],
                    rhs=wt2[:, :, n0 + 1],
                    start=True,
                    stop=True,
                    tile_position=(0, 64),
                )
            nc.vector.scalar_tensor_tensor(
                osb[:, g8 * 8 : (g8 + 1) * 8, :],
                po,
                1.0,
                ebs[:, nb, g8 * 8 : (g8 + 1) * 8, :],
                ALU.mult,
                ALU.add,
            )

        for nb in range(NB):
            xgt2, wt2 = blocks.pop(nb)
            osb = osbp.tile([128, 64, C], BF16, tag="osb")
            tnb = nb + 1 if nb >= 1 and nb + 1 < NB else None
            if tnb is not None:
                blocks[tnb] = start_block(tnb)
            for g8 in range(8):
                emit_node_group(nb, xgt2, wt2, osb, g8)
                if tnb is not None:
                    for b4 in (g8 * 2, g8 * 2 + 1):
                        emit_transpose_group(tnb, blocks[tnb][0], b4)
            for j in range(2):
                nc.sync.dma_start(
                    outd[:, nb, j, :, :], osb[j * 64 : (j + 1) * 64, :, :]
                )
    nc.compile()
    return nc


def prep_in_maps(x, node_embedding, weights_pool, bias_pool, cfg=None):
    """Host-side packing. Returns per-core input dicts for the SPMD kernel."""
    cfg = cfg or FULL_CFG
    B, C, N, GP, M = cfg["B"], cfg["C"], cfg["N"], cfg["GP"], cfg["M"]
    ncores = cfg["NCORES"]
    G = N // ncores
    MCP = M // 256
    NB = GP // 128
    BG = 8
    BCC = B // BG
    KI = 2 * C
    BF = ml_dtypes.bfloat16
    F8 = ml_dtypes.float8_e4m3

    x = np.asarray(x, np.float32)
    E = np.asarray(node_embedding, np.float32)
    Wp = np.asarray(weights_pool, np.float32)
    bp = np.asarray(bias_pool, np.float32)

    # padded global node order: 8 blocks of GP (G real + pad)
    x_t = np.ascontiguousarray(x.transpose(1, 0, 2))  # (N, B, C)
    xp0 = np.zeros((M, B, C), np.float32)
    ep0 = np.zeros((M, E.shape[1]), np.float32)
    real = np.zeros(M, bool)
    for g in range(ncores):
        xp0[g * GP : g * GP + G] = x_t[g * G : (g + 1) * G]
        ep0[g * GP : g * GP + G] = E[g * G : (g + 1) * G]
        real[g * GP : g * GP + G] = True

    # A8: normalized, scaled adjacency (fp8), mirrors the device's bf16 path
    z = ep0 @ ep0.T
    A_bf = np.maximum(np.exp(z, dtype=np.float32), 1.0).astype(BF).astype(np.float32)
    r = A_bf[real].sum(axis=0)  # col sums over real source rows
    A8full = A_bf * (ASCALE / r)[None, :]
    A8full[~real] = 0.0
    A8full = A8full.astype(F8)

    # x fp8 in DoubleRow-paired layout [128, bcc, mcp, j, bg*c] (same all cores)
    x8q = xp0.astype(F8)
    x8 = np.ascontiguousarray(
        x8q.reshape(MCP, 2, 128, BCC, BG, C)
        .transpose(2, 3, 0, 1, 4, 5)
        .reshape(128, BCC, MCP, 2, BG * C)
    )

    # per-node weights / bias (f32 einsum on host, cast bf16)
    Wfull = np.einsum("nd,dkio->nkio", E, Wp).astype(np.float32)  # (N,2,C,C)
    Wpad = np.zeros((M, 2, C, C), np.float32)
    ebias = np.zeros((M, C), np.float32)
    for g in range(ncores):
        Wpad[g * GP : g * GP + G] = Wfull[g * G : (g + 1) * G]
        ebias[g * GP : g * GP + G] = E[g * G : (g + 1) * G] @ bp

    in_maps = []
    for c in range(ncores):
        csl = slice(c * GP, (c + 1) * GP)
        if SWI:
            t = A8full[:, csl].reshape(MCP, 2, 128, NB, 128)[:, :, :, :, ::-1]
            a8c = np.ascontiguousarray(
                t.transpose(2, 0, 3, 4, 1).reshape(128, MCP, NB, 256)
            )
        else:
            a8c = np.ascontiguousarray(
                A8full[:, csl].reshape(MCP, 2, 128, GP).transpose(2, 0, 1, 3)
            )
        xpn = np.ascontiguousarray(
            xp0[csl].reshape(NB, 128, B, C).transpose(1, 0, 2, 3)
        ).astype(BF)
        wt = np.ascontiguousarray(
            Wpad[csl]
            .reshape(NB, 128, 2, C, C)
            .transpose(2, 3, 0, 4, 1)
            .reshape(KI, NB, C, 128)
        ).astype(BF)
        ebc = ebias[csl].reshape(NB, 64, 2, C)  # (nb, pair, j, o)
        ebd = np.zeros((128, NB, 64, C), np.float32)
        for j in range(2):
            ebd[j * 64 : (j + 1) * 64] = ebc[None, :, :, j, :]
        in_maps.append(
            {
                "x8d": x8,
                "a8d": a8c,
                "xpnd": xpn,
                "wtd": wt,
                "ebd": ebd.astype(BF),
            }
        )
    return in_maps


def unpack_output(outs, cfg=None):
    cfg = cfg or FULL_CFG
    B, C, GP, N = cfg["B"], cfg["C"], cfg["GP"], cfg["N"]
    ncores = cfg["NCORES"]
    G = N // ncores
    NB = GP // 128
    full = []
    for c in range(ncores):
        o = np.asarray(outs[c]["out_sh"]).astype(np.float32)  # [B, NB, 2, 64, C]
        o = o.transpose(0, 1, 3, 2, 4).reshape(B, GP, C)  # n = nb*128 + 2*pg + j
        full.append(o[:, :G, :])
    return np.concatenate(full, axis=1)


_NC_CACHE = {}
TRACE = False
LAST = None


def _get_nc(cfg_key, cfg):
    if cfg_key not in _NC_CACHE:
        _NC_CACHE[cfg_key] = build_nc(cfg)
    return _NC_CACHE[cfg_key]


def kernel(x, node_embedding, weights_pool, bias_pool):
    from concourse.bass_utils import run_bass_kernel_spmd

    cfg = dict(FULL_CFG)
    ncores = cfg["NCORES"]
    nc = _get_nc(("v2", cfg["GP"], cfg["M"], cfg["B"]), cfg)
    in_maps = prep_in_maps(x, node_embedding, weights_pool, bias_pool, cfg)

    global LAST
    res = run_bass_kernel_spmd(nc, in_maps, list(range(ncores)), trace=TRACE)
    LAST = res
    return unpack_output(res.results, cfg).astype(np.float32)
